# revision 6
# baseline (speedup 1.0000x reference)
"""AM sign-quantize hamming kernel, v2.

logit[b, c] = (D + sum_d sign(q[b,d]) * sign(am[c,d])) / 2

Per-core structure (8-way batch-parallel, am class-sharded + all-gathered):
  - am: 128 padded class rows per core; cast-load bf16, sign {0,1} (4x DVE),
    identity-transpose on PE, psum-evac on Act with bias -0.5 -> fp8 +-0.5,
    all-gather, regather into s-major saT [128, 8, KT, 128] (1x DMA).
  - q: 4 pair-tiles of 256 rows loaded PAIR-INTERLEAVED (rows 2p,2p+1 on
    partition p).  fp8-tiles: f32->fp8e4 casting DMA (halves DMA bytes),
    RAW values transposed 256 rows/pass by a DoubleRow identity matmul
    against I256 [128,2,256] (halves PE transpose cycles), sign folded
    into the DVE psum-evac ((x>0)-0.5 -> +-0.5).  bf16-tiles: bf16 cast
    load, 4x-rate DVE sign to {0,1} bits, std identity transpose, Act
    evac with bias -0.5.
  - main matmul: fp8 DoubleRow, 2 chains of 512 cols (4 shards each),
    psum = dot/4; logit = 2*psum + D/2 exactly, evac to int16.
"""

import sys

if "/opt/trn_rl_repo" not in sys.path:
    sys.path.insert(0, "/opt/trn_rl_repo")

import numpy as np

from concourse import bacc, bass, masks, mybir
from concourse.bass_utils import run_bass_kernel_spmd
from concourse.tile import TileContext

B, D, C = 8192, 10240, 1000
NCORES = 8
BS = B // NCORES  # 1024 batch rows per core
CP = 128  # padded am rows per core (8*128 = 1024 >= 1000)

F32 = mybir.dt.float32
BF16 = mybir.dt.bfloat16
FP8 = mybir.dt.float8e4
I16 = mybir.dt.int16

import os

KT = D // 128  # 80 k-tiles
DCH = int(os.environ.get("DCH", "2560"))  # load/sign/transpose chunk along D
NCH = D // DCH  # 4
KCH = DCH // 128  # 20 k-tiles per chunk

N_FP8_TILES = int(os.environ.get("N_FP8_TILES", "4"))
PS_T_BUFS = int(os.environ.get("PS_T_BUFS", "4"))
PS_MM_BUFS = int(os.environ.get("PS_MM_BUFS", "4"))
SINGLE_GATHER = os.environ.get("SINGLE_GATHER", "0") == "1"

gt = mybir.AluOpType.is_gt
add = mybir.AluOpType.add
sub = mybir.AluOpType.subtract
mult = mybir.AluOpType.mult
copyf = mybir.ActivationFunctionType.Copy
DR = mybir.MatmulPerfMode.DoubleRow


def build_nc() -> bass.Bass:
    nc = bacc.Bacc(None, target_bir_lowering=False, num_devices=NCORES)
    q_ext = nc.declare_dram_parameter("query", [BS, D], F32, isOutput=False)
    am_ext = nc.declare_dram_parameter("am_weight", [CP, D], F32, isOutput=False)
    out_ext = nc.declare_dram_parameter("out", [BS, C], I16, isOutput=True)

    with TileContext(nc) as tc:
        with (
            tc.tile_pool(name="const", bufs=1) as constp,
            tc.tile_pool(name="sat", bufs=1) as satp,
            tc.tile_pool(name="dram", bufs=1, space="DRAM") as dramp,
            tc.tile_pool(name="amload", bufs=2) as amload,
            tc.tile_pool(name="q8", bufs=3) as q8p,
            tc.tile_pool(name="qbf", bufs=1 if N_FP8_TILES == 4 else 2) as qbfp,
            tc.tile_pool(name="qt", bufs=2) as qtp,
            tc.tile_pool(name="outp", bufs=2) as outp,
            tc.tile_pool(name="h0", bufs=17 if N_FP8_TILES == 4 else 9) as h0p,
            tc.tile_pool(name="ps_t", bufs=PS_T_BUFS, space="PSUM") as ps_t,
            tc.tile_pool(name="ps_mm", bufs=PS_MM_BUFS, space="PSUM") as ps_mm,
        ):
            ident = constp.tile([128, 128], BF16)
            masks.make_identity(nc, ident[:])
            # I256[p, ko, n] = 1 iff n == 2p+ko : transposes 256
            # pair-interleaved rows per DoubleRow pass
            i256 = constp.tile([128, 2, 256], FP8)
            nc.vector.memset(i256[:], 0.0)
            for ko in range(2):
                nc.vector.tensor_scalar(
                    i256[:, ko, ko::2], ident[:], 0.0, None, add
                )

            saT = satp.tile([128, NCORES, KT, 128], FP8)  # s-major, 80K/part
            saTs = satp.tile([128, KT, 128], FP8)  # this core's slice
            KH = KT // 2
            b_in0 = dramp.tile([128, KH, 128], FP8)
            b_in1 = dramp.tile([128, KH, 128], FP8)
            b_out0 = dramp.tile([NCORES, 128, KH, 128], FP8, addr_space="Shared")
            b_out1 = dramp.tile([NCORES, 128, KH, 128], FP8, addr_space="Shared")

            # ---- phase A: am slice -> sign -> transpose -> all-gather -----
            # std-path 2-bank psum groups over the 20 k-tiles of a chunk
            GROUPS = (4,) * (KCH // 4)

            def am_chunk(ch):
                    a = amload.tile([128, DCH], BF16, tag="am")
                    nc.gpsimd.dma_start(
                        out=a[:], in_=am_ext[:, ch * DCH : (ch + 1) * DCH]
                    )
                    nc.vector.tensor_scalar(a[:], a[:], 0.0, None, gt)
                    kk = 0
                    for gsz in GROUPS:
                        pt = ps_t.tile([128, 4, 128], F32, tag="ps_t")
                        for j in range(gsz):
                            nc.tensor.matmul(
                                pt[:, j, :],
                                a[:, (kk + j) * 128 : (kk + j + 1) * 128],
                                ident[:],
                                start=(j == 0),
                                stop=(j == gsz - 1),
                                skip_group_check=True,
                            )
                        kbase = ch * KCH + kk
                        # Act-signed q kt-blocks hold +-1, compensated here
                        # with +-0.25 so products are +-0.25 everywhere
                        odd = (kbase // 4) % 2 == 1
                        nc.scalar.activation(
                            saTs[:, kbase : kbase + gsz, :],
                            pt[:, :gsz, :],
                            copyf,
                            bias=-0.25 if odd else -0.5,
                            scale=0.5 if odd else 1.0,
                        )
                        kk += gsz

            with tc.high_priority():
                for ch in range(NCH):
                    am_chunk(ch)
                for h, (bi, bo) in enumerate(((b_in0, b_out0), (b_in1, b_out1))):
                    nc.sync.dma_start(
                        out=bi[:], in_=saTs[:, h * KH : (h + 1) * KH, :]
                    )
                    nc.gpsimd.collective_compute(
                        "AllGather",
                        mybir.AluOpType.bypass,
                        replica_groups=[list(range(NCORES))],
                        ins=[bi[:].opt()],
                        outs=[bo[:].opt()],
                    )
                    for s in range(NCORES):
                        nc.sync.dma_start(
                            out=saT[:, s, h * KH : (h + 1) * KH, :],
                            in_=bo[s],
                        )

            # ---- phase B: q pair-tiles ------------------------------------
            for mt in range(4):  # pair-tiles of 256 rows
                b0 = mt * 256
                use8 = mt < N_FP8_TILES
                if use8:
                    qT = qtp.tile([128, KT, 256], FP8, tag="qt")
                else:
                    qT = qtp.tile([128, KT, 2, 128], FP8, tag="qt")
                for ch in range(NCH):
                    src = q_ext[
                        b0 : b0 + 256, ch * DCH : (ch + 1) * DCH
                    ].rearrange("(p two) d -> p two d", two=2)
                    if use8:
                        # raw fp8 cast load; sign folds into the psum evac
                        qf = q8p.tile([128, 2, DCH], FP8, tag="q8")
                        nc.gpsimd.dma_start(out=qf[:], in_=src)
                        # DR identity transpose, 4 k-tiles (2 banks) per group
                        for g in range(KCH // 2):
                            pt = ps_t.tile([128, 2, 256], F32, tag="ps_t")
                            for j in range(2):
                                kk = g * 2 + j
                                nc.tensor.matmul(
                                    pt[:, j, :],
                                    qf[:, :, kk * 128 : (kk + 1) * 128],
                                    i256[:],
                                    start=(j == 0),
                                    stop=(j == 1),
                                    skip_group_check=True,
                                    perf_mode=DR,
                                )
                            kbase = ch * KCH + g * 2
                            if (kbase // 4) % 2 == 0:
                                nc.vector.tensor_scalar(
                                    qT[:, kbase : kbase + 2, :],
                                    pt[:],
                                    0.0,
                                    0.5,
                                    gt,
                                    sub,
                                )
                            else:
                                nc.scalar.activation(
                                    qT[:, kbase : kbase + 2, :],
                                    pt[:],
                                    mybir.ActivationFunctionType.Sign,
                                )
                    else:
                        qf = qbfp.tile([128, 2, DCH], BF16, tag="qbf")
                        nc.gpsimd.dma_start(out=qf[:], in_=src)
                        nc.vector.tensor_scalar(qf[:], qf[:], 0.0, None, gt)
                        for ko in range(2):
                            kk = 0
                            for gsz in GROUPS:
                                pt = ps_t.tile([128, 4, 128], F32, tag="ps_t")
                                for j in range(gsz):
                                    nc.tensor.matmul(
                                        pt[:, j, :],
                                        qf[:, ko, (kk + j) * 128 : (kk + j + 1) * 128],
                                        ident[:],
                                        start=(j == 0),
                                        stop=(j == gsz - 1),
                                        skip_group_check=True,
                                    )
                                kbase = ch * KCH + kk
                                nc.scalar.activation(
                                    qT[:, kbase : kbase + gsz, ko, :],
                                    pt[:, :gsz, :],
                                    copyf,
                                    bias=-0.5,
                                )
                                kk += gsz

                # ---- main matmuls: per 128-row block, 2 chains of 512 ----
                # Each chain is split into two kt-half accumulations so the
                # first half only depends on the first all-gather half: the
                # h0 partial (2*psum + D/2, an exact small int) parks in SBUF
                # f32 and returns as the activation bias of the final evac.
                ot = outp.tile([128, 2, 1024], I16, tag="outp")

                def chain_mms(pm, qT, mb, chain, kplo, kphi):
                    for kp in range(kplo, kphi):
                        if use8:
                            lhsT = qT[
                                :, 2 * kp : 2 * kp + 2, mb * 128 : mb * 128 + 128
                            ]
                        else:
                            lhsT = qT[:, 2 * kp : 2 * kp + 2, mb, :]
                        for s in range(4):
                            # start zeroes the whole bank: only the first
                            # matmul of the bank sets it; stop only on the
                            # very last.
                            nc.tensor.matmul(
                                pm[:, s * 128 : (s + 1) * 128],
                                lhsT,
                                saT[:, chain * 4 + s, 2 * kp : 2 * kp + 2, :],
                                start=(kp == kplo and s == 0),
                                stop=(kp == kphi - 1 and s == 3),
                                skip_group_check=True,
                                perf_mode=DR,
                            )

                for mb in range(2):
                    pms = []
                    for chain in range(2):
                        pm = ps_mm.tile([128, 512], F32, tag="ps_mm")
                        pms.append(pm)
                        chain_mms(pm, qT, mb, chain, 0, KT // 2)
                    # logit = 2*psum + D/2 exactly (psum = dot/4)
                    for chain in range(2):
                        dst = ot[:, mb, chain * 512 : (chain + 1) * 512]
                        if (mt * 2 + mb + chain) % 2 == 0:
                            nc.scalar.activation(
                                dst, pms[chain][:], copyf, bias=float(D) / 2,
                                scale=2.0,
                            )
                        else:
                            nc.vector.tensor_scalar(
                                dst, pms[chain][:], 2.0, float(D) / 2, mult, add
                            )
                for mb in range(2):
                    if use8:
                        rows = out_ext[b0 + mb * 128 : b0 + (mb + 1) * 128, :]
                    else:
                        # std path leaves batch pair-interleaved: row 2j+mb
                        rows = out_ext[b0 + mb : b0 + 256 : 2, :]
                    nc.sync.dma_start(out=rows, in_=ot[:, mb, 0:C])

    nc.compile()
    return nc


_NC = None


def kernel(query: np.ndarray, am_weight: np.ndarray) -> np.ndarray:
    global _NC
    if _NC is None:
        _NC = build_nc()
    query = np.ascontiguousarray(query, dtype=np.float32)
    am_weight = np.ascontiguousarray(am_weight, dtype=np.float32)
    assert query.shape == (B, D), query.shape
    assert am_weight.shape == (C, D), am_weight.shape
    am_pad = np.zeros((NCORES * CP, D), dtype=np.float32)
    am_pad[:C] = am_weight
    in_maps = [
        {
            "query": query[i * BS : (i + 1) * BS],
            "am_weight": am_pad[i * CP : (i + 1) * CP],
        }
        for i in range(NCORES)
    ]
    res = run_bass_kernel_spmd(_NC, in_maps, core_ids=list(range(NCORES)))
    return np.concatenate(
        [res.results[i]["out"].astype(np.float32) for i in range(NCORES)], axis=0
    )


if __name__ == "__main__":
    q = np.random.randn(B, D).astype(np.float32)
    a = np.random.randn(C, D).astype(np.float32)
    out = kernel(q, a)
    sq = np.where(q > 0, 1.0, -1.0).astype(np.float32)
    sa = np.where(a > 0, 1.0, -1.0).astype(np.float32)
    ref = (D + sq @ sa.T) * 0.5
    err = np.abs(out - ref).max()
    print("max abs err:", err)
